# revision 14
# baseline (speedup 1.0000x reference)
"""Trainium2 Bass kernel for nn_DenseBayesian (dense + hard LWTA grouped argmax mask).

Computes out = x @ W.T + b, then per group of U=4 output units keeps only the
argmax unit (others zeroed). Data-parallel over 8 NeuronCores along the row axis.

v5 design (compact output, lane-blocked layout, f32r matmul):
  - W's columns are permuted host-side so the device sees lane-major order
    (col' = j*128 + h for original col h*4 + j). Group lanes then occupy
    contiguous 128-col blocks, enabling cheap per-block ops.
  - Matmul modes:
    "r": single-pass float32r (hw-internal split, 1 cyc/row) rel ~7e-3
    "e": 2-pass fp16 xh@(wh+wl)                              rel ~9.2e-3
    "a": 1-pass fp16                                         rel ~1.1e-2
  - The LWTA mask is never materialized. The winner index is packed into the
    low 2 mantissa bits of each logit ((bits & -4) | (3-j), one fused
    tensor_scalar per lane block, 2x DVE mode), then a 2-level pairwise-max
    tree yields winner value + index in one fp32 word per group. Host decodes
    and scatters into the dense [N, 512] output. Device output is 4x smaller
    than the dense result.
  - Per macro-tile: PE matmuls -> Act copies PSUM->SBUF -> DVE tags + maxes
    -> DMA out.

Self-contained: hardcodes the problem shapes; needs numpy + the concourse
runtime available on the host.
"""
import numpy as np

import concourse.bass as bass
import concourse.mybir as mybir
import concourse.tile as tile
from concourse import bacc
from concourse.bass_utils import run_bass_kernel_spmd

f32 = mybir.dt.float32
f32r = mybir.dt.float32r
f16 = mybir.dt.float16
i32 = mybir.dt.int32

N = 262144
DIN = 256
DOUT = 512
U = 4
NCORES = 8
ROWS = N // NCORES          # 32768 rows per core
MACRO = 1024                # rows per macro-tile (8 psum sections of 128 rows)
SEC = MACRO // 128          # 128-row sections per macro-tile
PSEC = 4                    # sections per psum tile (tile = [P, PSEC*DOUT])
P = 128
KC = DIN // P               # k chunks
G = DOUT // U               # groups per row (128)

# lane-major column permutation: new col j*G + h  <- old col h*U + j
_PERM = np.arange(DOUT).reshape(G, U).T.reshape(-1)

BIAS96 = 96.0               # exponent-pinning bias: logits + 96 in [64, 128)
ULP1 = float(np.float32(2.0 ** -17))  # 1 ulp at exponent 6


def build_program(n_macros: int, with_bias: bool, mode: str = "r"):
    """One NeuronCore program: n_macros macro-tiles of MACRO rows each.

    mode: "r" 1-pass float32r; "e" 2-pass fp16; "a" 1-pass fp16.
    All emit the compact packed-max output [nm, P, SEC*G].
    """
    assert mode in ("a", "e", "r")
    nc = bacc.Bacc("TRN2", target_bir_lowering=False)

    xdt = f32r if mode == "r" else f16
    x_d = nc.dram_tensor("x", [n_macros, P, KC, MACRO], xdt, kind="ExternalInput")
    w_d = nc.dram_tensor("w", [P, KC, DOUT], xdt, kind="ExternalInput")
    if mode == "e":
        wl_d = nc.dram_tensor("wl", [P, KC, DOUT], f16, kind="ExternalInput")
    if with_bias:
        bh_d = nc.dram_tensor("bh", [1, DOUT], f16, kind="ExternalInput")
        bl_d = nc.dram_tensor("bl", [1, DOUT], f16, kind="ExternalInput")
    out_d = nc.dram_tensor("out", [n_macros, P, SEC * G], f32,
                           kind="ExternalOutput")

    with tile.TileContext(nc) as tc:
        with tc.tile_pool(name="wpool", bufs=1) as wpool, \
             tc.tile_pool(name="xpool", bufs=4) as xpool, \
             tc.tile_pool(name="upool", bufs=4) as upool, \
             tc.tile_pool(name="tpool", bufs=4) as tpool, \
             tc.tile_pool(name="mpool", bufs=4) as mpool, \
             tc.tile_pool(name="pspool", bufs=2, space="PSUM") as pspool:

            w = wpool.tile([P, KC, DOUT], xdt)
            nc.sync.dma_start(w[:], w_d[:])
            if mode == "e":
                wl = wpool.tile([P, KC, DOUT], f16)
                nc.sync.dma_start(wl[:], wl_d[:])
            if with_bias:
                bh = wpool.tile([1, DOUT], f16)
                nc.sync.dma_start(bh[:], bh_d[:])
                bl = wpool.tile([1, DOUT], f16)
                nc.sync.dma_start(bl[:], bl_d[:])
                ones = wpool.tile([1, P], f16)
                nc.gpsimd.memset(ones[:], 1.0)
            b96 = wpool.tile([P, 1], f32)
            nc.gpsimd.memset(b96[:], BIAS96)

            for mt in range(n_macros):
                x_t = xpool.tile([P, KC, MACRO], xdt, tag="x")
                nc.sync.dma_start(x_t[:], x_d[mt, :, :, :])

                # logits + 96 for MACRO rows, lane-major: u[p, (s j h)].
                # +96 pins every value into [64, 128) (|logit| < 30), so all
                # share exponent 6 (ulp 2^-17) and float order == positive
                # order; "set tag bit k" = "add k*2^-17", exact after the
                # low-bit clear.
                u = upool.tile([P, SEC * DOUT], f32)

                for half in range(SEC // PSEC):
                    ps = pspool.tile([P, PSEC * DOUT], f32, tag="ps")
                    for sq in range(PSEC):
                        s = half * PSEC + sq
                        acc = ps[:, sq * DOUT:(sq + 1) * DOUT]
                        rs = slice(s * P, (s + 1) * P)
                        mms = []
                        if with_bias:
                            mms.append((ones[:, :], bh[:, :]))
                            mms.append((ones[:, :], bl[:, :]))
                        for c in range(KC):
                            mms.append((x_t[:, c, rs], w[:, c, :]))
                        if mode == "e":
                            for c in range(KC):
                                mms.append((x_t[:, c, rs], wl[:, c, :]))
                        last = len(mms) - 1
                        for i, (lhsT, rhs) in enumerate(mms):
                            nc.tensor.matmul(acc, lhsT, rhs, start=(i == 0),
                                             stop=(i == last))
                    nc.scalar.activation(
                        u[:, half * PSEC * DOUT:(half + 1) * PSEC * DOUT],
                        ps[:], mybir.ActivationFunctionType.Relu,
                        bias=b96[:, 0:1])

                # clear the low 2 mantissa bits (tag space), 2x DVE mode
                ui = u[:].bitcast(i32)
                nc.vector.tensor_scalar(ui[:], ui[:], -4, None,
                                        op0=mybir.AluOpType.bitwise_and)

                # 2-level pairwise max over lanes (j = 2a+b), winner tags
                # fused into the maxes as exact ulp adds (bit0: b=0 won,
                # bit1: a=0 won).
                uv = u[:].rearrange("p (s a b h) -> p s a b h", a=2, b=2, h=G)
                t1 = tpool.tile([P, SEC * 2 * G], f32)
                t1v = t1[:].rearrange("p (s a h) -> p s a h", a=2, h=G)
                nc.vector.scalar_tensor_tensor(
                    t1v, uv[:, :, :, 0, :], ULP1, uv[:, :, :, 1, :],
                    op0=mybir.AluOpType.add, op1=mybir.AluOpType.max)
                m = mpool.tile([P, SEC * G], f32)
                mv = m[:].rearrange("p (s h) -> p s h", h=G)
                nc.vector.scalar_tensor_tensor(
                    mv, t1v[:, :, 0, :], 2 * ULP1, t1v[:, :, 1, :],
                    op0=mybir.AluOpType.add, op1=mybir.AluOpType.max)

                # issue the store from the idle GpSimd queue to keep the SP
                # sequencer off the critical path
                nc.gpsimd.dma_start(out_d[mt, :, :], m[:])

    nc.compile()
    return nc


_programs: dict = {}


def _get_program(n_macros: int, with_bias: bool, mode: str = "r"):
    key = (n_macros, with_bias, mode)
    if key not in _programs:
        _programs[key] = build_program(n_macros, with_bias, mode)
    return _programs[key]


def _split_fp16(a: np.ndarray):
    hi = a.astype(np.float16)
    lo = (a - hi.astype(np.float32)).astype(np.float16)
    return hi, lo


def _pack_b(b: np.ndarray):
    """[DOUT] fp32 -> (hi, lo) [1, DOUT] fp16, lane-major permuted."""
    return _split_fp16(np.ascontiguousarray(
        b.astype(np.float32)[_PERM].reshape(1, DOUT)))


def _tile_x(a: np.ndarray, n_macros: int):
    """[rows, DIN] -> [n_macros, P, KC, MACRO] (keeps dtype)."""
    at = np.ascontiguousarray(a.T)                      # [DIN, rows]
    at = at.reshape(KC, P, n_macros, MACRO)             # [c, p, mt, r]
    return np.ascontiguousarray(at.transpose(2, 1, 0, 3))


def _tile_w(a: np.ndarray):
    """[DIN, DOUT] lane-major -> [P, KC, DOUT] (keeps dtype)."""
    return np.ascontiguousarray(a.reshape(KC, P, DOUT).transpose(1, 0, 2))


def _pack_inputs(x: np.ndarray, W: np.ndarray, b: np.ndarray, with_bias: bool,
                 n_macros: int, mode: str):
    """Build the per-core input maps (x sharded over rows, W replicated)."""
    wT = np.ascontiguousarray(W.astype(np.float32).T[:, _PERM])  # [DIN, DOUT]
    if mode == "r":
        base = {"w": _tile_w(wT)}
    else:
        wh = wT.astype(np.float16)
        base = {"w": _tile_w(wh)}
        if mode == "e":
            wl = (wT - wh.astype(np.float32)).astype(np.float16)
            base["wl"] = _tile_w(wl)
    if with_bias:
        bhi, blo = _pack_b(b)
        base["bh"] = bhi
        base["bl"] = blo

    in_maps = []
    for i in range(NCORES):
        xs = x[i * ROWS:(i + 1) * ROWS]
        im = dict(base)
        if mode == "r":
            im["x"] = _tile_x(xs.astype(np.float32), n_macros)
        else:
            im["x"] = _tile_x(xs.astype(np.float16), n_macros)
        in_maps.append(im)
    return in_maps


def _decode_core(marr: np.ndarray, n_macros: int) -> np.ndarray:
    """[nm, P, SEC*G] packed winners -> dense [ROWS, DOUT]."""
    bits = np.ascontiguousarray(marr).view(np.uint32)
    j = (3 - (bits & 3)).astype(np.intp)
    # exact Sterbenz subtraction undoes the +96 exponent-pinning bias
    v = (bits & ~np.uint32(3)).view(np.float32) - np.float32(BIAS96)
    # row = mt*MACRO + s*128 + p, group h; out col = h*U + j
    jr = j.reshape(n_macros, P, SEC, G).transpose(0, 2, 1, 3)
    vr = v.reshape(n_macros, P, SEC, G).transpose(0, 2, 1, 3)
    z = np.zeros((n_macros, SEC, P, G, U), np.float32)
    np.put_along_axis(z, jr[..., None], vr[..., None], axis=4)
    return z.reshape(ROWS, DOUT)


def kernel(x: np.ndarray, W: np.ndarray, b: np.ndarray) -> np.ndarray:
    x = np.asarray(x, dtype=np.float32)
    W = np.asarray(W, dtype=np.float32)
    b = np.asarray(b, dtype=np.float32)
    assert x.shape == (N, DIN) and W.shape == (DOUT, DIN) and b.shape == (DOUT,)

    with_bias = bool(np.any(b))
    n_macros = ROWS // MACRO
    mode = "r"
    nc = _get_program(n_macros, with_bias, mode)
    in_maps = _pack_inputs(x, W, b, with_bias, n_macros, mode)
    res = run_bass_kernel_spmd(nc, in_maps, list(range(NCORES)))
    return np.concatenate(
        [_decode_core(res.results[i]["out"], n_macros) for i in range(NCORES)],
        axis=0)


# revision 15
# speedup vs baseline: 1.0178x; 1.0178x over previous
"""Trainium2 Bass kernel for nn_DenseBayesian (dense + hard LWTA grouped argmax mask).

Computes out = x @ W.T + b, then per group of U=4 output units keeps only the
argmax unit (others zeroed). Data-parallel over 8 NeuronCores along the row axis.

v8 design (compact output, lane-blocked layout, f32r matmul, ramped tiles):
  - W's columns are permuted host-side so the device sees lane-major order
    (col' = j*128 + h for original col h*4 + j). Group lanes then occupy
    contiguous 128-col blocks, enabling cheap per-block ops.
  - Matmul modes:
    "r": single-pass float32r (hw-internal split, 1 cyc/row) rel ~7.9e-3
    "e": 2-pass fp16 xh@(wh+wl)                              rel ~9.2e-3
    "a": 1-pass fp16                                         rel ~1.1e-2
  - The LWTA mask is never materialized. Logits are biased +96 during the
    PSUM->SBUF drain (free on the Act engine), pinning every value into
    [64, 128) -- one shared exponent, ulp = 2^-17, float order == value
    order. After one fused low-2-bit clear (2x DVE mode), the winner index
    accumulates INSIDE the two pairwise-max ops: scalar_tensor_tensor
    (add 1ulp, max) marks "b=0 won" in bit0, (add 2ulp, max) marks "a=0 won"
    in bit1 (ulp adds are exact: the bits are clear and the exponent is
    pinned). One fp32 word per group carries winner value + index; the host
    decodes and scatters into the dense [N, 512] output. Device output is
    4x smaller than the dense result. Ties break toward the smaller index,
    matching argmax.
  - Tile sizes ramp 128..1024 at the start and back down at the end so the
    PE->Act->DVE->DMA pipeline fills/drains quickly; DVE (the throughput
    limiter) runs gap-free in between.

Self-contained: hardcodes the problem shapes; needs numpy + the concourse
runtime available on the host.
"""
import numpy as np

import concourse.bass as bass
import concourse.mybir as mybir
import concourse.tile as tile
from concourse import bacc
from concourse.bass_utils import run_bass_kernel_spmd

f32 = mybir.dt.float32
f32r = mybir.dt.float32r
f16 = mybir.dt.float16
i32 = mybir.dt.int32

N = 262144
DIN = 256
DOUT = 512
U = 4
NCORES = 8
ROWS = N // NCORES          # 32768 rows per core
P = 128
KC = DIN // P               # k chunks
G = DOUT // U               # groups per row (128)
NSG = ROWS // P             # 128-row sections per core (256)

# tile schedule: ramp up, steady 1024-row tiles, ramp down
_TILES = [128, 128, 256, 512] + [1024] * 30 + [512, 256, 128, 128]
assert sum(_TILES) == ROWS
PSEC = 4                    # max sections per psum tile

# lane-major column permutation: new col j*G + h  <- old col h*U + j
_PERM = np.arange(DOUT).reshape(G, U).T.reshape(-1)

BIAS96 = 96.0               # exponent-pinning bias: logits + 96 in [64, 128)
ULP1 = float(np.float32(2.0 ** -17))  # 1 ulp at exponent 6


def build_program(with_bias: bool, mode: str = "r"):
    """One NeuronCore program over the ramped tile schedule.

    mode: "r" 1-pass float32r; "e" 2-pass fp16; "a" 1-pass fp16.
    Emits the compact packed-max output [P, NSG*G].
    """
    assert mode in ("a", "e", "r")
    nc = bacc.Bacc("TRN2", target_bir_lowering=False)

    xdt = f32r if mode == "r" else f16
    x_d = nc.dram_tensor("x", [P, KC, ROWS], xdt, kind="ExternalInput")
    w_d = nc.dram_tensor("w", [P, KC, DOUT], xdt, kind="ExternalInput")
    if mode == "e":
        wl_d = nc.dram_tensor("wl", [P, KC, DOUT], f16, kind="ExternalInput")
    if with_bias:
        bh_d = nc.dram_tensor("bh", [1, DOUT], f16, kind="ExternalInput")
        bl_d = nc.dram_tensor("bl", [1, DOUT], f16, kind="ExternalInput")
    out_d = nc.dram_tensor("out", [P, NSG * G], f32, kind="ExternalOutput")

    with tile.TileContext(nc) as tc:
        with tc.tile_pool(name="wpool", bufs=1) as wpool, \
             tc.tile_pool(name="xpool", bufs=4) as xpool, \
             tc.tile_pool(name="upool", bufs=4) as upool, \
             tc.tile_pool(name="tpool", bufs=4) as tpool, \
             tc.tile_pool(name="mpool", bufs=4) as mpool, \
             tc.tile_pool(name="pspool", bufs=2, space="PSUM") as pspool:

            w = wpool.tile([P, KC, DOUT], xdt)
            nc.sync.dma_start(w[:], w_d[:])
            if mode == "e":
                wl = wpool.tile([P, KC, DOUT], f16)
                nc.sync.dma_start(wl[:], wl_d[:])
            if with_bias:
                bh = wpool.tile([1, DOUT], f16)
                nc.sync.dma_start(bh[:], bh_d[:])
                bl = wpool.tile([1, DOUT], f16)
                nc.sync.dma_start(bl[:], bl_d[:])
                ones = wpool.tile([1, P], f16)
                nc.gpsimd.memset(ones[:], 1.0)
            b96 = wpool.tile([P, 1], f32)
            nc.gpsimd.memset(b96[:], BIAS96)

            row0 = 0
            for rows in _TILES:
                sec = rows // P
                x_t = xpool.tile([P, KC, rows], xdt, tag="x")
                nc.sync.dma_start(x_t[:], x_d[:, :, row0:row0 + rows])

                # logits + 96, lane-major: u[p, (s j h)]
                u = upool.tile([P, sec * DOUT], f32, tag="u")

                for p0 in range(0, sec, PSEC):
                    psec = min(PSEC, sec - p0)
                    ps = pspool.tile([P, psec * DOUT], f32, tag="ps")
                    for sq in range(psec):
                        s = p0 + sq
                        acc = ps[:, sq * DOUT:(sq + 1) * DOUT]
                        rs = slice(s * P, (s + 1) * P)
                        mms = []
                        if with_bias:
                            mms.append((ones[:, :], bh[:, :]))
                            mms.append((ones[:, :], bl[:, :]))
                        for c in range(KC):
                            mms.append((x_t[:, c, rs], w[:, c, :]))
                        if mode == "e":
                            for c in range(KC):
                                mms.append((x_t[:, c, rs], wl[:, c, :]))
                        last = len(mms) - 1
                        for i, (lhsT, rhs) in enumerate(mms):
                            nc.tensor.matmul(acc, lhsT, rhs, start=(i == 0),
                                             stop=(i == last))
                    nc.scalar.activation(
                        u[:, p0 * DOUT:(p0 + psec) * DOUT],
                        ps[:], mybir.ActivationFunctionType.Relu,
                        bias=b96[:, 0:1])

                # clear the low 2 mantissa bits (tag space), 2x DVE mode
                ui = u[:].bitcast(i32)
                nc.vector.tensor_scalar(ui[:], ui[:], -4, None,
                                        op0=mybir.AluOpType.bitwise_and)

                # 2-level pairwise max over lanes (j = 2a+b), winner tags
                # fused into the maxes as exact ulp adds (bit0: b=0 won,
                # bit1: a=0 won).
                uv = u[:].rearrange("p (s a b h) -> p s a b h", a=2, b=2, h=G)
                t1 = tpool.tile([P, sec * 2 * G], f32, tag="t1")
                t1v = t1[:].rearrange("p (s a h) -> p s a h", a=2, h=G)
                nc.vector.scalar_tensor_tensor(
                    t1v, uv[:, :, :, 0, :], ULP1, uv[:, :, :, 1, :],
                    op0=mybir.AluOpType.add, op1=mybir.AluOpType.max)
                m = mpool.tile([P, sec * G], f32, tag="m")
                mv = m[:].rearrange("p (s h) -> p s h", h=G)
                nc.vector.scalar_tensor_tensor(
                    mv, t1v[:, :, 0, :], 2 * ULP1, t1v[:, :, 1, :],
                    op0=mybir.AluOpType.add, op1=mybir.AluOpType.max)

                sg0 = row0 // P
                nc.gpsimd.dma_start(out_d[:, sg0 * G:(sg0 + sec) * G], m[:])
                row0 += rows

    nc.compile()
    return nc


_programs: dict = {}


def _get_program(with_bias: bool, mode: str = "r"):
    key = (with_bias, mode)
    if key not in _programs:
        _programs[key] = build_program(with_bias, mode)
    return _programs[key]


def _split_fp16(a: np.ndarray):
    hi = a.astype(np.float16)
    lo = (a - hi.astype(np.float32)).astype(np.float16)
    return hi, lo


def _pack_b(b: np.ndarray):
    """[DOUT] fp32 -> (hi, lo) [1, DOUT] fp16, lane-major permuted."""
    return _split_fp16(np.ascontiguousarray(
        b.astype(np.float32)[_PERM].reshape(1, DOUT)))


def _tile_x(a: np.ndarray):
    """[rows, DIN] -> [P, KC, ROWS] (keeps dtype)."""
    at = np.ascontiguousarray(a.T)                      # [DIN, rows]
    return np.ascontiguousarray(at.reshape(KC, P, ROWS).transpose(1, 0, 2))


def _tile_w(a: np.ndarray):
    """[DIN, DOUT] lane-major -> [P, KC, DOUT] (keeps dtype)."""
    return np.ascontiguousarray(a.reshape(KC, P, DOUT).transpose(1, 0, 2))


def _pack_inputs(x: np.ndarray, W: np.ndarray, b: np.ndarray, with_bias: bool,
                 mode: str):
    """Build the per-core input maps (x sharded over rows, W replicated)."""
    wT = np.ascontiguousarray(W.astype(np.float32).T[:, _PERM])  # [DIN, DOUT]
    if mode == "r":
        base = {"w": _tile_w(wT)}
    else:
        wh = wT.astype(np.float16)
        base = {"w": _tile_w(wh)}
        if mode == "e":
            wl = (wT - wh.astype(np.float32)).astype(np.float16)
            base["wl"] = _tile_w(wl)
    if with_bias:
        bhi, blo = _pack_b(b)
        base["bh"] = bhi
        base["bl"] = blo

    in_maps = []
    for i in range(NCORES):
        xs = x[i * ROWS:(i + 1) * ROWS]
        im = dict(base)
        if mode == "r":
            im["x"] = _tile_x(xs.astype(np.float32))
        else:
            im["x"] = _tile_x(xs.astype(np.float16))
        in_maps.append(im)
    return in_maps


def _decode_core(marr: np.ndarray) -> np.ndarray:
    """[P, NSG*G] packed winners -> dense [ROWS, DOUT]."""
    bits = np.ascontiguousarray(marr).view(np.uint32)
    j = (3 - (bits & 3)).astype(np.intp)
    # exact Sterbenz subtraction undoes the +96 exponent-pinning bias
    v = (bits & ~np.uint32(3)).view(np.float32) - np.float32(BIAS96)
    # marr[p, sg*G + h] <-> row sg*128 + p, group h; out col = h*U + j
    jr = j.reshape(P, NSG, G).transpose(1, 0, 2)
    vr = v.reshape(P, NSG, G).transpose(1, 0, 2)
    z = np.zeros((NSG, P, G, U), np.float32)
    np.put_along_axis(z, jr[..., None], vr[..., None], axis=3)
    return z.reshape(ROWS, DOUT)


def kernel(x: np.ndarray, W: np.ndarray, b: np.ndarray) -> np.ndarray:
    x = np.asarray(x, dtype=np.float32)
    W = np.asarray(W, dtype=np.float32)
    b = np.asarray(b, dtype=np.float32)
    assert x.shape == (N, DIN) and W.shape == (DOUT, DIN) and b.shape == (DOUT,)

    with_bias = bool(np.any(b))
    mode = "r"
    nc = _get_program(with_bias, mode)
    in_maps = _pack_inputs(x, W, b, with_bias, mode)
    res = run_bass_kernel_spmd(nc, in_maps, list(range(NCORES)))
    return np.concatenate(
        [_decode_core(res.results[i]["out"]) for i in range(NCORES)], axis=0)


# revision 17
# speedup vs baseline: 1.0213x; 1.0034x over previous
"""Trainium2 Bass kernel for nn_DenseBayesian (dense + hard LWTA grouped argmax mask).

Computes out = x @ W.T + b, then per group of U=4 output units keeps only the
argmax unit (others zeroed). Data-parallel over 8 NeuronCores along the row axis.

v8 design (compact output, lane-blocked layout, f32r matmul, ramped tiles):
  - W's columns are permuted host-side so the device sees lane-major order
    (col' = j*128 + h for original col h*4 + j). Group lanes then occupy
    contiguous 128-col blocks, enabling cheap per-block ops.
  - Matmul modes:
    "r": single-pass float32r (hw-internal split, 1 cyc/row) rel ~7.9e-3
    "e": 2-pass fp16 xh@(wh+wl)                              rel ~9.2e-3
    "a": 1-pass fp16                                         rel ~1.1e-2
  - The LWTA mask is never materialized. Logits are biased +96 during the
    PSUM->SBUF drain (free on the Act engine), pinning every value into
    [64, 128) -- one shared exponent, ulp = 2^-17, float order == value
    order. After one fused low-2-bit clear (2x DVE mode), the winner index
    accumulates INSIDE the two pairwise-max ops: scalar_tensor_tensor
    (add 1ulp, max) marks "b=0 won" in bit0, (add 2ulp, max) marks "a=0 won"
    in bit1 (ulp adds are exact: the bits are clear and the exponent is
    pinned). One fp32 word per group carries winner value + index; the host
    decodes and scatters into the dense [N, 512] output. Device output is
    4x smaller than the dense result. Ties break toward the smaller index,
    matching argmax.
  - Tile sizes ramp 128..1024 at the start and back down at the end so the
    PE->Act->DVE->DMA pipeline fills/drains quickly; DVE (the throughput
    limiter) runs gap-free in between.

Self-contained: hardcodes the problem shapes; needs numpy + the concourse
runtime available on the host.
"""
import numpy as np

import concourse.bass as bass
import concourse.mybir as mybir
import concourse.tile as tile
from concourse import bacc
from concourse.bass_utils import run_bass_kernel_spmd

f32 = mybir.dt.float32
f32r = mybir.dt.float32r
f16 = mybir.dt.float16
i32 = mybir.dt.int32

N = 262144
DIN = 256
DOUT = 512
U = 4
NCORES = 8
ROWS = N // NCORES          # 32768 rows per core
P = 128
KC = DIN // P               # k chunks
G = DOUT // U               # groups per row (128)
NSG = ROWS // P             # 128-row sections per core (256)

# tile schedule: ramp up, steady 1024-row tiles, ramp down
_TILES = [128, 128, 256, 512] + [1024] * 30 + [512, 256, 128, 128]
assert sum(_TILES) == ROWS
PSEC = 2                    # max sections per psum tile

# lane-major column permutation: new col j*G + h  <- old col h*U + j
_PERM = np.arange(DOUT).reshape(G, U).T.reshape(-1)

BIAS96 = 96.0               # exponent-pinning bias: logits + 96 in [64, 128)
ULP1 = float(np.float32(2.0 ** -17))  # 1 ulp at exponent 6


def build_program(with_bias: bool, mode: str = "r"):
    """One NeuronCore program over the ramped tile schedule.

    mode: "r" 1-pass float32r; "e" 2-pass fp16; "a" 1-pass fp16.
    Emits the compact packed-max output [P, NSG*G].
    """
    assert mode in ("a", "e", "r")
    nc = bacc.Bacc("TRN2", target_bir_lowering=False)

    xdt = f32r if mode == "r" else f16
    x_d = nc.dram_tensor("x", [P, KC, ROWS], xdt, kind="ExternalInput")
    w_d = nc.dram_tensor("w", [P, KC, DOUT], xdt, kind="ExternalInput")
    if mode == "e":
        wl_d = nc.dram_tensor("wl", [P, KC, DOUT], f16, kind="ExternalInput")
    if with_bias:
        bh_d = nc.dram_tensor("bh", [1, DOUT], f16, kind="ExternalInput")
        bl_d = nc.dram_tensor("bl", [1, DOUT], f16, kind="ExternalInput")
    out_d = nc.dram_tensor("out", [P, NSG * G], f32, kind="ExternalOutput")

    with tile.TileContext(nc) as tc:
        with tc.tile_pool(name="wpool", bufs=1) as wpool, \
             tc.tile_pool(name="xpool", bufs=4) as xpool, \
             tc.tile_pool(name="upool", bufs=4) as upool, \
             tc.tile_pool(name="tpool", bufs=4) as tpool, \
             tc.tile_pool(name="mpool", bufs=4) as mpool, \
             tc.tile_pool(name="pspool", bufs=4, space="PSUM") as pspool:

            w = wpool.tile([P, KC, DOUT], xdt)
            for c in range(KC):
                nc.sync.dma_start(w[:, c, :], w_d[:, c, :])
            if mode == "e":
                wl = wpool.tile([P, KC, DOUT], f16)
                nc.sync.dma_start(wl[:], wl_d[:])
            if with_bias:
                bh = wpool.tile([1, DOUT], f16)
                nc.sync.dma_start(bh[:], bh_d[:])
                bl = wpool.tile([1, DOUT], f16)
                nc.sync.dma_start(bl[:], bl_d[:])
                ones = wpool.tile([1, P], f16)
                nc.gpsimd.memset(ones[:], 1.0)
            b96 = wpool.tile([P, 1], f32)
            nc.gpsimd.memset(b96[:], BIAS96)

            row0 = 0
            for rows in _TILES:
                sec = rows // P
                x_t = xpool.tile([P, KC, rows], xdt, tag="x")
                nc.sync.dma_start(x_t[:], x_d[:, :, row0:row0 + rows])

                # logits + 96, lane-major: u[p, (s j h)]
                u = upool.tile([P, sec * DOUT], f32, tag="u")

                for p0 in range(0, sec, PSEC):
                    psec = min(PSEC, sec - p0)
                    ps = pspool.tile([P, psec * DOUT], f32, tag="ps")
                    for sq in range(psec):
                        s = p0 + sq
                        acc = ps[:, sq * DOUT:(sq + 1) * DOUT]
                        rs = slice(s * P, (s + 1) * P)
                        mms = []
                        if with_bias:
                            mms.append((ones[:, :], bh[:, :]))
                            mms.append((ones[:, :], bl[:, :]))
                        for c in range(KC):
                            mms.append((x_t[:, c, rs], w[:, c, :]))
                        if mode == "e":
                            for c in range(KC):
                                mms.append((x_t[:, c, rs], wl[:, c, :]))
                        last = len(mms) - 1
                        for i, (lhsT, rhs) in enumerate(mms):
                            nc.tensor.matmul(acc, lhsT, rhs, start=(i == 0),
                                             stop=(i == last))
                    nc.scalar.activation(
                        u[:, p0 * DOUT:(p0 + psec) * DOUT],
                        ps[:], mybir.ActivationFunctionType.Relu,
                        bias=b96[:, 0:1])

                # clear the low 2 mantissa bits (tag space), 2x DVE mode
                ui = u[:].bitcast(i32)
                nc.vector.tensor_scalar(ui[:], ui[:], -4, None,
                                        op0=mybir.AluOpType.bitwise_and)

                # 2-level pairwise max over lanes (j = 2a+b), winner tags
                # fused into the maxes as exact ulp adds (bit0: b=0 won,
                # bit1: a=0 won).
                uv = u[:].rearrange("p (s a b h) -> p s a b h", a=2, b=2, h=G)
                t1 = tpool.tile([P, sec * 2 * G], f32, tag="t1")
                t1v = t1[:].rearrange("p (s a h) -> p s a h", a=2, h=G)
                nc.vector.scalar_tensor_tensor(
                    t1v, uv[:, :, :, 0, :], ULP1, uv[:, :, :, 1, :],
                    op0=mybir.AluOpType.add, op1=mybir.AluOpType.max)
                m = mpool.tile([P, sec * G], f32, tag="m")
                mv = m[:].rearrange("p (s h) -> p s h", h=G)
                nc.vector.scalar_tensor_tensor(
                    mv, t1v[:, :, 0, :], 2 * ULP1, t1v[:, :, 1, :],
                    op0=mybir.AluOpType.add, op1=mybir.AluOpType.max)

                sg0 = row0 // P
                nc.gpsimd.dma_start(out_d[:, sg0 * G:(sg0 + sec) * G], m[:])
                row0 += rows

    nc.compile()
    return nc


_programs: dict = {}


def _get_program(with_bias: bool, mode: str = "r"):
    key = (with_bias, mode)
    if key not in _programs:
        _programs[key] = build_program(with_bias, mode)
    return _programs[key]


def _split_fp16(a: np.ndarray):
    hi = a.astype(np.float16)
    lo = (a - hi.astype(np.float32)).astype(np.float16)
    return hi, lo


def _pack_b(b: np.ndarray):
    """[DOUT] fp32 -> (hi, lo) [1, DOUT] fp16, lane-major permuted."""
    return _split_fp16(np.ascontiguousarray(
        b.astype(np.float32)[_PERM].reshape(1, DOUT)))


def _tile_x(a: np.ndarray):
    """[rows, DIN] -> [P, KC, ROWS] (keeps dtype)."""
    at = np.ascontiguousarray(a.T)                      # [DIN, rows]
    return np.ascontiguousarray(at.reshape(KC, P, ROWS).transpose(1, 0, 2))


def _tile_w(a: np.ndarray):
    """[DIN, DOUT] lane-major -> [P, KC, DOUT] (keeps dtype)."""
    return np.ascontiguousarray(a.reshape(KC, P, DOUT).transpose(1, 0, 2))


def _pack_inputs(x: np.ndarray, W: np.ndarray, b: np.ndarray, with_bias: bool,
                 mode: str):
    """Build the per-core input maps (x sharded over rows, W replicated)."""
    wT = np.ascontiguousarray(W.astype(np.float32).T[:, _PERM])  # [DIN, DOUT]
    if mode == "r":
        base = {"w": _tile_w(wT)}
    else:
        wh = wT.astype(np.float16)
        base = {"w": _tile_w(wh)}
        if mode == "e":
            wl = (wT - wh.astype(np.float32)).astype(np.float16)
            base["wl"] = _tile_w(wl)
    if with_bias:
        bhi, blo = _pack_b(b)
        base["bh"] = bhi
        base["bl"] = blo

    in_maps = []
    for i in range(NCORES):
        xs = x[i * ROWS:(i + 1) * ROWS]
        im = dict(base)
        if mode == "r":
            im["x"] = _tile_x(xs.astype(np.float32))
        else:
            im["x"] = _tile_x(xs.astype(np.float16))
        in_maps.append(im)
    return in_maps


def _decode_core(marr: np.ndarray) -> np.ndarray:
    """[P, NSG*G] packed winners -> dense [ROWS, DOUT]."""
    bits = np.ascontiguousarray(marr).view(np.uint32)
    j = (3 - (bits & 3)).astype(np.intp)
    # exact Sterbenz subtraction undoes the +96 exponent-pinning bias
    v = (bits & ~np.uint32(3)).view(np.float32) - np.float32(BIAS96)
    # marr[p, sg*G + h] <-> row sg*128 + p, group h; out col = h*U + j
    jr = j.reshape(P, NSG, G).transpose(1, 0, 2)
    vr = v.reshape(P, NSG, G).transpose(1, 0, 2)
    z = np.zeros((NSG, P, G, U), np.float32)
    np.put_along_axis(z, jr[..., None], vr[..., None], axis=3)
    return z.reshape(ROWS, DOUT)


def kernel(x: np.ndarray, W: np.ndarray, b: np.ndarray) -> np.ndarray:
    x = np.asarray(x, dtype=np.float32)
    W = np.asarray(W, dtype=np.float32)
    b = np.asarray(b, dtype=np.float32)
    assert x.shape == (N, DIN) and W.shape == (DOUT, DIN) and b.shape == (DOUT,)

    with_bias = bool(np.any(b))
    mode = "r"
    nc = _get_program(with_bias, mode)
    in_maps = _pack_inputs(x, W, b, with_bias, mode)
    res = run_bass_kernel_spmd(nc, in_maps, list(range(NCORES)))
    return np.concatenate(
        [_decode_core(res.results[i]["out"]) for i in range(NCORES)], axis=0)
